# revision 3
# baseline (speedup 1.0000x reference)
"""Trainium2 Bass kernel for channel-wise weighted reduction + capped relu.

Computes out[b, s] = capped_relu(sum_c x[b,c,s] * W[c,s] + bias[s]) for
x [64, 256, 4096] f32, W [256, 4096] f32, bias [4096] f32.

Sharding: data-parallel over batch across 8 NeuronCores (8 batches/core),
weights + bias replicated. No cross-core communication.

Per-core pipeline:
  - DMA x[b] as one SBUF tile [128ch, 2*4096] (two 2 MiB transfers).
  - DVE: y = x * W elementwise (in-place), one [128, 4096] op per c-half.
  - PE:  channel reduction as matmul with the products stationary and a
    ones[128,1] vector moving: out column psum[:, b*32+t] accumulates the
    two c-halves. This streams only 1 moving row per matmul (fp32 moving
    rows cost 4 cyc/row, so keeping the data on the stationary side wins).
  - Epilogue (tiny, 128-partition ops on [128, 256]):
      tb = psum + bias ; mask = is_le(max(tb,0),1) ; o = max(tb,0)*mask
  - PE transpose per batch -> [32t, 128s] so DRAM stores are 512B runs.
"""

import numpy as np

B, C, S = 64, 256, 4096
NCORES = 8
BPC = B // NCORES          # batches per core
NT = S // 128              # 32 s-chunks of 128
H = C // 128               # 2 channel halves

_cache = {}


def _build_nc():
    import concourse.bacc as bacc
    import concourse.bass as bass
    import concourse.mybir as mybir
    from concourse.masks import make_identity
    from concourse.tile import TileContext

    f32 = mybir.dt.float32
    Alu = mybir.AluOpType

    nc = bacc.Bacc(
        "TRN2",
        target_bir_lowering=False,
        debug=False,
        num_devices=NCORES,
    )

    x_d = nc.dram_tensor("x", [BPC, C, S], f32, kind="ExternalInput").ap()
    w_d = nc.dram_tensor("weights", [C, S], f32, kind="ExternalInput").ap()
    b_d = nc.dram_tensor("bias", [S], f32, kind="ExternalInput").ap()
    o_d = nc.dram_tensor("out", [BPC, S], f32, kind="ExternalOutput").ap()

    with TileContext(nc) as tc:
        with (
            tc.tile_pool(name="consts", bufs=1) as cpool,
            tc.tile_pool(name="xbuf", bufs=3) as xpool,
            tc.tile_pool(name="epi", bufs=1) as epool,
            tc.tile_pool(name="ps", bufs=1, space="PSUM") as ppool,
        ):
            # Replicated weights, both halves side by side: [:, h*S:(h+1)*S]
            w_t = cpool.tile([128, H * S], f32, name="w_t")
            for h in range(H):
                nc.sync.dma_start(w_t[:, h * S:(h + 1) * S], w_d[h * 128:(h + 1) * 128, :])

            ones_t = cpool.tile([128, 1], f32, name="ones_t")
            nc.vector.memset(ones_t[:], 1.0)

            ident = cpool.tile([128, 128], f32, name="ident")
            make_identity(nc, ident[:])

            # bias_rep[p, b*NT+t] = bias[t*128+p], replicated for all b
            bias_rep = cpool.tile([128, BPC * NT], f32, name="bias_rep")
            bias_pt = b_d.rearrange("(t p) -> p t", p=128)
            for b in range(BPC):
                nc.sync.dma_start(bias_rep[:, b * NT:(b + 1) * NT], bias_pt)

            # psum_main[q, b*NT+t] = sum_c x[b, c, t*128+q] * W[c, t*128+q]
            # The two c-half matmuls of a column form one accumulation group
            # and are emitted ADJACENT: a start=True clears has_written for
            # the whole bank, so groups must not interleave (completed
            # columns' data persists — only the accumulate bits are cleared).
            psum_main = ppool.tile([128, BPC * NT], f32, name="psum_main")
            psum_tr = ppool.tile([32, BPC * 128], f32, name="psum_tr")

            for b in range(BPC):
                x_t = xpool.tile([128, H * S], f32, name="x_t", tag="x")
                for h in range(H):
                    nc.sync.dma_start(
                        x_t[:, h * S:(h + 1) * S], x_d[b, h * 128:(h + 1) * 128, :]
                    )
                for h in range(H):
                    off = h * S
                    nc.vector.tensor_tensor(
                        x_t[:, off:off + S],
                        x_t[:, off:off + S],
                        w_t[:, off:off + S],
                        Alu.mult,
                    )
                for t in range(NT):
                    col = b * NT + t
                    for h in range(H):
                        nc.tensor.matmul(
                            psum_main[:, col:col + 1],
                            x_t[:, h * S + t * 128: h * S + (t + 1) * 128],
                            ones_t[:, 0:1],
                            start=(h == 0),
                            stop=(h == H - 1),
                        )

            # Epilogue: capped relu on [128, 256]
            tb = epool.tile([128, BPC * NT], f32, name="tb")
            nc.vector.tensor_tensor(tb[:], psum_main[:], bias_rep[:], Alu.add)
            msk = epool.tile([128, BPC * NT], f32, name="msk")
            nc.vector.tensor_scalar(msk[:], tb[:], 0.0, 1.0, Alu.max, Alu.is_le)
            o_sb = epool.tile([128, BPC * NT], f32, name="o_sb")
            nc.vector.scalar_tensor_tensor(
                o_sb[:], tb[:], 0.0, msk[:], Alu.max, Alu.mult
            )

            # Transpose each batch's [128q, 32t] block to [32t, 128q] so the
            # store writes 512B-contiguous runs.
            for b in range(BPC):
                nc.tensor.transpose(
                    psum_tr[0:32, b * 128:(b + 1) * 128],
                    o_sb[:, b * NT:(b + 1) * NT],
                    ident[:],
                )
            out_sb = epool.tile([32, BPC * 128], f32, name="out_sb")
            nc.scalar.activation(
                out_sb[:], psum_tr[:], mybir.ActivationFunctionType.Copy
            )

            # out_sb[t, b*128+q] -> out[b, t*128+q]
            out_view = o_d.rearrange("b (t p) -> t b p", p=128)
            in_view = out_sb[:].rearrange("t (b p) -> t b p", p=128)
            nc.sync.dma_start(out_view, in_view)

    nc.compile()
    return nc


def kernel(x: np.ndarray, weights: np.ndarray, bias: np.ndarray) -> np.ndarray:
    from concourse.bass_utils import run_bass_kernel_spmd

    if "nc" not in _cache:
        _cache["nc"] = _build_nc()
    nc = _cache["nc"]

    x = np.ascontiguousarray(x, dtype=np.float32)
    weights = np.ascontiguousarray(weights, dtype=np.float32)
    bias = np.ascontiguousarray(bias, dtype=np.float32)

    in_maps = [
        {
            "x": x[i * BPC:(i + 1) * BPC],
            "weights": weights,
            "bias": bias,
        }
        for i in range(NCORES)
    ]
    res = run_bass_kernel_spmd(nc, in_maps, core_ids=list(range(NCORES)))
    return np.concatenate([res.results[i]["out"] for i in range(NCORES)], axis=0)


# revision 11
# speedup vs baseline: 1.3374x; 1.3374x over previous
"""Trainium2 Bass kernel for channel-wise weighted reduction + capped relu.

Computes out[b, s] = capped_relu(sum_c x[b,c,s] * W[c,s] + bias[s]) for
x [64, 256, 4096] f32, W [256, 4096] f32, bias [4096] f32.

Sharding: data-parallel over batch across 8 NeuronCores (8 batches/core),
weights + bias replicated. No cross-core communication.

Per-core pipeline:
  - DMA x[b] as one SBUF tile [128ch, 2*4096] (two 2 MiB transfers).
  - DVE: y = x * W elementwise (in-place), one [128, 4096] op per c-half.
  - PE:  channel reduction as matmul with ones[128,1] STATIONARY (loaded
    once, 1 column) and the products MOVING: out row psum[b, chunk] =
    ones.T @ y_chunk. fp32 moving rows cost 4 cyc/row; for FOLD_BATCHES
    of the 8 batches the two c-halves are pre-summed on DVE (one extra
    [128,4096] add) which halves that batch's PE stream — the knob
    balances DVE vs PE occupancy.
  - Epilogue on [8, 4096]: tb = psum + bias ; mask = is_le(max(tb,0),1) ;
    o = max(tb,0)*mask ; direct row-major store.
"""

import numpy as np

B, C, S = 64, 256, 4096
NCORES = 8
BPC = B // NCORES          # batches per core
NJ = S // 512              # 8 psum-bank chunks of 512
H = C // 128               # 2 channel halves

_cache = {}


def _build_nc(fold_batches=4, use_f32r=False):
    import concourse.bacc as bacc
    import concourse.bass as bass
    import concourse.mybir as mybir
    from concourse.tile import TileContext

    f32 = mybir.dt.float32
    Alu = mybir.AluOpType

    nc = bacc.Bacc(
        "TRN2",
        target_bir_lowering=False,
        debug=False,
        num_devices=NCORES,
    )

    x_d = nc.dram_tensor("x", [BPC, C, S], f32, kind="ExternalInput").ap()
    w_d = nc.dram_tensor("weights", [C, S], f32, kind="ExternalInput").ap()
    b_d = nc.dram_tensor("bias", [S], f32, kind="ExternalInput").ap()
    o_d = nc.dram_tensor("out", [BPC, S], f32, kind="ExternalOutput").ap()

    with TileContext(nc) as tc:
        with (
            tc.tile_pool(name="consts", bufs=1) as cpool,
            tc.tile_pool(name="xbuf", bufs=2) as xpool,
            tc.tile_pool(name="stg", bufs=2) as spool,
            tc.tile_pool(name="epi", bufs=1) as epool,
            tc.tile_pool(name="ps", bufs=1, space="PSUM") as ppool,
        ):
            # Replicated weights, both halves side by side: [:, h*S:(h+1)*S]
            w_t = cpool.tile([128, H * S], f32, name="w_t")
            for h in range(H):
                nc.sync.dma_start(w_t[:, h * S:(h + 1) * S], w_d[h * 128:(h + 1) * 128, :])

            ones_t = cpool.tile([128, 1], f32, name="ones_t")
            nc.vector.memset(ones_t[:], 1.0)

            # bias broadcast to the 8 output rows
            bias_bc = cpool.tile([BPC, S], f32, name="bias_bc")
            for b in range(BPC):
                nc.sync.dma_start(bias_bc[b:b + 1, :], b_d[None, :])

            # PE output rows must sit on 32-aligned partitions: batch b uses
            # psum partition 32*(b%4) via an explicit tile_position column
            # group, so 4 batches can be in flight in PSUM at once. Each
            # finished row is ACT-copied to out_acc[b, :] in SBUF.
            psum_big = ppool.tile([128, S], f32, name="psum_big")
            out_acc = epool.tile([BPC, S], f32, name="out_acc")

            for b in range(BPC):
                g = 32 * (b % 4)
                x_t = xpool.tile([128, H * S], f32, name="x_t", tag="x")
                for h in range(H):
                    nc.sync.dma_start(
                        x_t[:, h * S:(h + 1) * S], x_d[b, h * 128:(h + 1) * 128, :]
                    )
                for h in range(H):
                    off = h * S
                    nc.vector.tensor_tensor(
                        x_t[:, off:off + S],
                        x_t[:, off:off + S],
                        w_t[:, off:off + S],
                        Alu.mult,
                    )
                fold = b < fold_batches
                if fold:
                    # z = y_h0 + y_h1 in place -> halves the PE stream
                    nc.vector.tensor_tensor(
                        x_t[:, 0:S], x_t[:, 0:S], x_t[:, S:H * S], Alu.add
                    )
                nhalf = 1 if fold else H
                for j in range(NJ):
                    for h in range(nhalf):
                        rhs = x_t[:, h * S + j * 512: h * S + (j + 1) * 512]
                        lhsT = ones_t[:, 0:1]
                        if use_f32r:
                            rhs = rhs.bitcast(mybir.dt.float32r)
                            lhsT = lhsT.bitcast(mybir.dt.float32r)
                        nc.tensor.matmul(
                            psum_big[g:g + 1, j * 512:(j + 1) * 512],
                            lhsT,
                            rhs,
                            start=(h == 0),
                            stop=(h == nhalf - 1),
                            tile_position=(0, g),
                        )
                # Drain this batch's row: compute engines can only address
                # 32-aligned SBUF partition windows, so ACT-copy the psum row
                # (base 32g, legal) to a partition-0 staging row, then pack
                # it onto partition b of out_acc with an SBUF->SBUF DMA
                # (DMA has no partition-alignment restriction).
                stg = spool.tile([1, S], f32, name="stg", tag="stg")
                nc.scalar.activation(
                    stg[:, :],
                    psum_big[g:g + 1, :],
                    mybir.ActivationFunctionType.Copy,
                )
                nc.sync.dma_start(out_acc[b:b + 1, :], stg[:, :])

            # Epilogue: capped relu on [8, 4096] in two s-halves, row store
            for s0 in (0, S // 2):
                sl = slice(s0, s0 + S // 2)
                tb = epool.tile([BPC, S // 2], f32, name="tb", tag="tb", bufs=1)
                nc.vector.tensor_tensor(tb[:], out_acc[:, sl], bias_bc[:, sl], Alu.add)
                msk = epool.tile([BPC, S // 2], f32, name="msk", tag="msk", bufs=1)
                nc.vector.tensor_scalar(msk[:], tb[:], 0.0, 1.0, Alu.max, Alu.is_le)
                o_sb = epool.tile([BPC, S // 2], f32, name="o_sb", tag="o", bufs=1)
                nc.vector.scalar_tensor_tensor(
                    o_sb[:], tb[:], 0.0, msk[:], Alu.max, Alu.mult
                )
                nc.sync.dma_start(o_d[:, sl], o_sb[:])

    nc.compile()
    return nc


def kernel(x: np.ndarray, weights: np.ndarray, bias: np.ndarray) -> np.ndarray:
    from concourse.bass_utils import run_bass_kernel_spmd

    if "nc" not in _cache:
        _cache["nc"] = _build_nc()
    nc = _cache["nc"]

    x = np.ascontiguousarray(x, dtype=np.float32)
    weights = np.ascontiguousarray(weights, dtype=np.float32)
    bias = np.ascontiguousarray(bias, dtype=np.float32)

    in_maps = [
        {
            "x": x[i * BPC:(i + 1) * BPC],
            "weights": weights,
            "bias": bias,
        }
        for i in range(NCORES)
    ]
    res = run_bass_kernel_spmd(nc, in_maps, core_ids=list(range(NCORES)))
    return np.concatenate([res.results[i]["out"] for i in range(NCORES)], axis=0)


# revision 14
# speedup vs baseline: 1.5805x; 1.1818x over previous
"""Trainium2 Bass kernel for channel-wise weighted reduction + capped relu.

Computes out[b, s] = capped_relu(sum_c x[b,c,s] * W[c,s] + bias[s]) for
x [64, 256, 4096] f32, W [256, 4096] f32, bias [4096] f32.

Sharding: data-parallel over batch across 8 NeuronCores (8 batches/core),
weights + bias replicated. No cross-core communication.

Per-core pipeline:
  - DMA x[b] as one SBUF tile [128ch, 2*4096] (two 2 MiB transfers).
  - DVE: y = x * W elementwise (in-place), one [128, 4096] op per c-half.
  - PE:  channel reduction as matmul with ones[128,1] STATIONARY (loaded
    once, 1 column) and the products MOVING: out row psum[b, chunk] =
    ones.T @ y_chunk. fp32 moving rows cost 4 cyc/row; for FOLD_BATCHES
    of the 8 batches the two c-halves are pre-summed on DVE (one extra
    [128,4096] add) which halves that batch's PE stream — the knob
    balances DVE vs PE occupancy.
  - Epilogue on [8, 4096]: tb = psum + bias ; mask = is_le(max(tb,0),1) ;
    o = max(tb,0)*mask ; direct row-major store.
"""

import numpy as np

B, C, S = 64, 256, 4096
NCORES = 8
BPC = B // NCORES          # batches per core
NJ = S // 512              # 8 psum-bank chunks of 512
H = C // 128               # 2 channel halves

_cache = {}


def _build_nc(fold_batches=4, use_f32r=False):
    import concourse.bacc as bacc
    import concourse.bass as bass
    import concourse.mybir as mybir
    from concourse.tile import TileContext

    f32 = mybir.dt.float32
    Alu = mybir.AluOpType

    nc = bacc.Bacc(
        "TRN2",
        target_bir_lowering=False,
        debug=False,
        num_devices=NCORES,
    )

    x_d = nc.dram_tensor("x", [BPC, C, S], f32, kind="ExternalInput").ap()
    w_d = nc.dram_tensor("weights", [C, S], f32, kind="ExternalInput").ap()
    b_d = nc.dram_tensor("bias", [S], f32, kind="ExternalInput").ap()
    o_d = nc.dram_tensor("out", [BPC, S], f32, kind="ExternalOutput").ap()

    with TileContext(nc) as tc:
        with (
            tc.tile_pool(name="consts", bufs=1) as cpool,
            tc.tile_pool(name="xbuf", bufs=2) as xpool,
            tc.tile_pool(name="stg", bufs=2) as spool,
            tc.tile_pool(name="epi", bufs=1) as epool,
            tc.tile_pool(name="ps", bufs=1, space="PSUM") as ppool,
        ):
            # Replicated weights, both halves side by side: [:, h*S:(h+1)*S]
            w_t = cpool.tile([128, H * S], f32, name="w_t")
            for h in range(H):
                nc.sync.dma_start(w_t[:, h * S:(h + 1) * S], w_d[h * 128:(h + 1) * 128, :])

            ones_t = cpool.tile([128, 1], f32, name="ones_t")
            nc.vector.memset(ones_t[:], 1.0)

            # bias broadcast to the 8 output rows
            bias_bc = cpool.tile([BPC, S], f32, name="bias_bc")
            for b in range(BPC):
                nc.sync.dma_start(bias_bc[b:b + 1, :], b_d[None, :])

            # PE output rows must sit on 32-aligned partitions, and a PSUM
            # bank being read (ACT drain) while the PE writes it serializes
            # the pipeline. Slot map: batch parity picks the bank half
            # (free-dim half), (b//2)%2 picks the row pair — consecutive
            # batches touch disjoint banks, so drains overlap next batch's
            # matmuls. Each batch's 4096-wide row lives as 2 half-rows:
            #   chunk j -> row 32*(2*((b//2)%2) + j//4),
            #             free offset (S//2)*(b%2) + (j%4)*512.
            psum_big = ppool.tile([128, S], f32, name="psum_big")
            out_acc = epool.tile([BPC, S], f32, name="out_acc")

            for b in range(BPC):
                hb = b % 2              # bank half (free-dim half)
                rp = (b // 2) % 2       # row pair
                x_t = xpool.tile([128, H * S], f32, name="x_t", tag="x")
                for h in range(H):
                    nc.sync.dma_start(
                        x_t[:, h * S:(h + 1) * S], x_d[b, h * 128:(h + 1) * 128, :]
                    )
                for h in range(H):
                    off = h * S
                    nc.vector.tensor_tensor(
                        x_t[:, off:off + S],
                        x_t[:, off:off + S],
                        w_t[:, off:off + S],
                        Alu.mult,
                    )
                fold = b < fold_batches
                if fold:
                    # z = y_h0 + y_h1 in place -> halves the PE stream
                    nc.vector.tensor_tensor(
                        x_t[:, 0:S], x_t[:, 0:S], x_t[:, S:H * S], Alu.add
                    )
                nhalf = 1 if fold else H
                for j in range(NJ):
                    row = 32 * (2 * rp + j // 4)
                    off = (S // 2) * hb + (j % 4) * 512
                    for h in range(nhalf):
                        rhs = x_t[:, h * S + j * 512: h * S + (j + 1) * 512]
                        lhsT = ones_t[:, 0:1]
                        if use_f32r:
                            rhs = rhs.bitcast(mybir.dt.float32r)
                            lhsT = lhsT.bitcast(mybir.dt.float32r)
                        nc.tensor.matmul(
                            psum_big[row:row + 1, off:off + 512],
                            lhsT,
                            rhs,
                            start=(h == 0),
                            stop=(h == nhalf - 1),
                            tile_position=(0, row),
                        )
                # Drain this batch's two half-rows: compute engines can only
                # address 32-aligned SBUF partition windows, so ACT-copy each
                # psum half-row to a partition-0 staging row, then pack it
                # onto partition b of out_acc with an SBUF->SBUF DMA
                # (DMA has no partition-alignment restriction).
                stg = spool.tile([1, S], f32, name="stg", tag="stg")
                for half in range(2):
                    row = 32 * (2 * rp + half)
                    off = (S // 2) * hb
                    nc.scalar.activation(
                        stg[:, half * (S // 2):(half + 1) * (S // 2)],
                        psum_big[row:row + 1, off:off + S // 2],
                        mybir.ActivationFunctionType.Copy,
                    )
                nc.sync.dma_start(out_acc[b:b + 1, :], stg[:, :])

            # Epilogue: capped relu on [8, 4096] in two s-halves, row store
            for s0 in (0, S // 2):
                sl = slice(s0, s0 + S // 2)
                tb = epool.tile([BPC, S // 2], f32, name="tb", tag="tb", bufs=1)
                nc.vector.tensor_tensor(tb[:], out_acc[:, sl], bias_bc[:, sl], Alu.add)
                msk = epool.tile([BPC, S // 2], f32, name="msk", tag="msk", bufs=1)
                nc.vector.tensor_scalar(msk[:], tb[:], 0.0, 1.0, Alu.max, Alu.is_le)
                o_sb = epool.tile([BPC, S // 2], f32, name="o_sb", tag="o", bufs=1)
                nc.vector.scalar_tensor_tensor(
                    o_sb[:], tb[:], 0.0, msk[:], Alu.max, Alu.mult
                )
                nc.sync.dma_start(o_d[:, sl], o_sb[:])

    nc.compile()
    return nc


def kernel(x: np.ndarray, weights: np.ndarray, bias: np.ndarray) -> np.ndarray:
    from concourse.bass_utils import run_bass_kernel_spmd

    if "nc" not in _cache:
        _cache["nc"] = _build_nc()
    nc = _cache["nc"]

    x = np.ascontiguousarray(x, dtype=np.float32)
    weights = np.ascontiguousarray(weights, dtype=np.float32)
    bias = np.ascontiguousarray(bias, dtype=np.float32)

    in_maps = [
        {
            "x": x[i * BPC:(i + 1) * BPC],
            "weights": weights,
            "bias": bias,
        }
        for i in range(NCORES)
    ]
    res = run_bass_kernel_spmd(nc, in_maps, core_ids=list(range(NCORES)))
    return np.concatenate([res.results[i]["out"] for i in range(NCORES)], axis=0)
